# revision 1
# baseline (speedup 1.0000x reference)
"""Deformable cross-attention kernel for 8 Trainium2 NeuronCores.

Data-parallel over batch N=8: core i processes batch element i.
Per-core pipeline:
  1. query -> PE transpose -> offsets/attn projection (fp32 matmul)
  2. DVE weight math: sampling locations, floors, clamps, border masks,
     softmax, fused bilinear*attn coefficients, gather window indices
  3. memory -> bf16 (cast in DMA) -> PE transpose -> bf16 matmul with
     W_value -> value stored in DRAM as head-pairs [4][16386, 64] bf16
  4. dma_gather: 64 calls (head, point, ycorner) of 1024 query indices,
     overlapping 4-row windows (512B elems, 256B step)
  5. DVE blend: gathered * coeff (3 live sub-rows), reduce, accumulate
  6. bias folding, PE transpose of sampled, f32r output projection
"""
import json
import os
import numpy as np
import ml_dtypes

N_B = 8
LQ = 1024
C = 256
NH = 8
NPT = 4
HD = 32
HW = 16384
GRID = 128  # H == W == 128
PAIR_ROWS = HW + 2  # 2 pad rows, zeroed


def _patch_compat(bass):
    """Split multi-wait instructions and sem-range-clears for this walrus."""
    if getattr(bass.Bass, "_dca_patched", False):
        return
    orig = bass.Bass.to_json_bytes

    def to_json_bytes(self):
        m = json.loads(orig(self))
        uid = 0
        sem_names = m.get("ant_sem_names") or {}
        for fn in m["functions"]:
            for bb in fn["blocks"]:
                out = []
                for inst in bb["instructions"]:
                    si = inst.get("sync_info")
                    waits = (si or {}).get("on_wait") or []
                    if len(waits) > 1:
                        for w in waits[:-1]:
                            uid += 1
                            out.append({
                                "debug": inst.get("debug", 0),
                                "engine": inst["engine"],
                                "ins": [], "outs": [],
                                "name": f"I-wsplit-{uid}",
                                "opcode": "EventSemaphore",
                                "sync_info": {"on_update": [], "on_wait": [w]},
                            })
                        si["on_wait"] = waits[-1:]
                    if (inst.get("opcode") == "ISA"
                            and inst.get("op_name") == "EVENT_SEMAPHORE_RANGE_CLEAR"):
                        d = inst["ant_dict"]
                        for sid in range(d["range_first"], d["range_last"] + 1):
                            uid += 1
                            out.append({
                                "debug": inst.get("debug", 0),
                                "engine": inst["engine"],
                                "ins": [], "outs": [],
                                "name": f"I-semclr-{uid}",
                                "opcode": "EventSemaphore",
                                "sync_info": {
                                    "on_wait": [],
                                    "on_update": [{
                                        "ant_name": sem_names.get(str(sid), f"sem{sid}"),
                                        "id": sid, "sync_type": "semaphore",
                                        "update_mode": "sem-wr-imm",
                                        "update_value": 0,
                                    }]},
                            })
                        continue
                    out.append(inst)
                bb["instructions"] = out
        return json.dumps(m).encode()

    bass.Bass.to_json_bytes = to_json_bytes
    bass.Bass._dca_patched = True


def _floor(nc, pool, mybir, x, shape, pfx):
    """Exact floor via double cast + is_gt correction (any cast rounding)."""
    F32 = mybir.dt.float32
    xi = pool.tile(shape, mybir.dt.int32, name=f"{pfx}_xi", tag=f"{pfx}_xi")
    nc.vector.tensor_copy(xi[:], x[:])
    xf = pool.tile(shape, F32, name=f"{pfx}_xf", tag=f"{pfx}_xf")
    nc.vector.tensor_copy(xf[:], xi[:])
    gt = pool.tile(shape, F32, name=f"{pfx}_gt", tag=f"{pfx}_gt")
    nc.vector.tensor_tensor(gt[:], xf[:], x[:], mybir.AluOpType.is_gt)
    fl = pool.tile(shape, F32, name=f"{pfx}_fl", tag=f"{pfx}_fl")
    nc.vector.tensor_tensor(fl[:], xf[:], gt[:], mybir.AluOpType.subtract)
    return fl


def build_program():
    import concourse.bass as bass
    import concourse.bacc as bacc
    import concourse.mybir as mybir
    import concourse.tile as tile
    from contextlib import ExitStack

    _patch_compat(bass)

    F32 = mybir.dt.float32
    F32R = mybir.dt.float32r
    BF16 = mybir.dt.bfloat16
    I16 = mybir.dt.int16
    TT = mybir.AluOpType
    ACTF = mybir.ActivationFunctionType

    nc = bacc.Bacc()

    # ---- external tensors ----
    query_e = nc.declare_dram_parameter("query", [LQ, C], F32, isOutput=False)
    memory_e = nc.declare_dram_parameter("memory", [HW, C], F32, isOutput=False)
    refpts_e = nc.declare_dram_parameter("refpts", [LQ, 2], F32, isOutput=False)
    w_value_e = nc.declare_dram_parameter("w_value", [C, C], BF16, isOutput=False)
    w_oa_e = nc.declare_dram_parameter("w_oa", [C, 96], F32, isOutput=False)
    b_oa_e = nc.declare_dram_parameter("b_oa", [128, 96], F32, isOutput=False)
    w_out_e = nc.declare_dram_parameter("w_out", [C, C], F32R, isOutput=False)
    b_out_e = nc.declare_dram_parameter("b_out", [128, C], F32, isOutput=False)
    b_val_e = nc.declare_dram_parameter("b_val", [128, C], F32, isOutput=False)
    ident_e = nc.declare_dram_parameter("ident", [128, 128], F32, isOutput=False)
    identb_e = nc.declare_dram_parameter("identb", [128, 128], BF16, isOutput=False)
    out_e = nc.declare_dram_parameter("out", [LQ, C], F32, isOutput=True)

    # internal DRAM: value as head-pairs [4][PAIR_ROWS, 64] bf16
    value_d = nc.dram_tensor("value_pairs", [4, PAIR_ROWS, 64], BF16)

    with tile.TileContext(nc) as tc, ExitStack() as ctx:
        cpool = ctx.enter_context(tc.tile_pool(name="const", bufs=1))
        qpool = ctx.enter_context(tc.tile_pool(name="qp", bufs=2))
        wpool = ctx.enter_context(tc.tile_pool(name="wm", bufs=1))
        mpool = ctx.enter_context(tc.tile_pool(name="mem", bufs=3))
        vpool = ctx.enter_context(tc.tile_pool(name="val", bufs=3))
        gpool = ctx.enter_context(tc.tile_pool(name="gat", bufs=4))
        spool = ctx.enter_context(tc.tile_pool(name="scr", bufs=2))
        psA = ctx.enter_context(tc.tile_pool(name="psA", bufs=3, space="PSUM"))
        psB = ctx.enter_context(tc.tile_pool(name="psB", bufs=2, space="PSUM"))
        _ps_n = [0]

        def ps_tr():  # [128,128] f32 transpose target, shared slots
            _ps_n[0] += 1
            return psA.tile([128, 128], F32, name=f"pstr{_ps_n[0]}", tag="pstr")

        def ps_trb():  # bf16 transpose target
            _ps_n[0] += 1
            return psA.tile([128, 128], BF16, name=f"psb{_ps_n[0]}", tag="psb")

        def ps_mm():  # [128,256] f32 matmul target
            _ps_n[0] += 1
            return psB.tile([128, C], F32, name=f"psmm{_ps_n[0]}", tag="psmm")

        ident = cpool.tile([128, 128], F32)
        nc.sync.dma_start(ident[:], ident_e[:])
        identb = cpool.tile([128, 128], BF16)
        nc.sync.dma_start(identb[:], identb_e[:])

        # ---------- 1. queryT + offsets/attn projection ----------
        # query [1024, 256] -> queryT [2][128, 1024]
        qT = [cpool.tile([128, LQ], F32, name=f"qT{i}", tag=f"qT{i}")
              for i in range(2)]
        for qc in range(8):
            qt = qpool.tile([128, C], F32)
            nc.sync.dma_start(qt[:], query_e[qc * 128:(qc + 1) * 128, :])
            for kc in range(2):
                pt = ps_tr()
                nc.tensor.transpose(pt[:], qt[:, kc * 128:(kc + 1) * 128], ident[:])
                nc.scalar.copy(qT[kc][:, qc * 128:(qc + 1) * 128], pt[:])

        w_oa = cpool.tile([128, 2, 96], F32)
        nc.sync.dma_start(w_oa[:], w_oa_e[:].rearrange("(k p) o -> p k o", k=2))
        b_oa = cpool.tile([128, 96], F32)
        nc.sync.dma_start(b_oa[:], b_oa_e[:])

        # off_all [128, 8, 96] natural layout (partition = q%128, qtop free)
        off_all = cpool.tile([128, 8, 96], F32)
        for qc in range(8):
            po = ps_mm()
            for kc in range(2):
                nc.tensor.matmul(po[:, 0:96], qT[kc][:, qc * 128:(qc + 1) * 128],
                                 w_oa[:, kc, :], start=(kc == 0), stop=(kc == 1))
            nc.vector.tensor_tensor(off_all[:, qc, :], po[:, 0:96], b_oa[:], TT.add)

        # refpts natural [128, 8, 2]
        refs = cpool.tile([128, 8, 2], F32)
        nc.sync.dma_start(
            refs[:], refpts_e[:].rearrange("(g p) t -> p g t", p=128))

        # ---------- 2. weight math ----------
        S8 = [128, 8, 32]      # (q%128, qtop, (h, pt))

        def view_off(comp):  # comp 0 = x, 1 = y -> [128, 8, 8, 4] strided view
            return off_all[:, :, comp:64 + comp].rearrange(
                "p g (h pt two) -> p g h pt two", h=8, two=2)[:, :, :, :, 0]

        wm = ctx.enter_context(tc.tile_pool(name="wmath", bufs=1))

        _wm_n = [0]

        def ttile():
            _wm_n[0] += 1
            nm = f"wmath{_wm_n[0]}"
            return wm.tile(S8, F32, name=nm, tag=nm)

        # --- x chain ---
        px = ttile()
        # px = (ref_x + ox/128) * 128 - 0.5   (matches reference rounding)
        nc.vector.tensor_scalar(px[:], view_off(0), 1.0 / GRID, None, TT.mult)
        nc.vector.tensor_tensor(
            px[:], px[:], refs[:, :, 0:1].broadcast_to(S8), TT.add)
        nc.vector.tensor_scalar(px[:], px[:], float(GRID), -0.5, TT.mult, TT.add)
        x0 = _floor(nc, wm, mybir, px, S8, "fx0")
        wx1 = ttile()
        nc.vector.tensor_tensor(wx1[:], px[:], x0[:], TT.subtract)
        wx0 = ttile()
        nc.vector.tensor_scalar(wx0[:], wx1[:], -1.0, 1.0, TT.mult, TT.add)
        ge0 = ttile()
        nc.vector.tensor_scalar(ge0[:], x0[:], 0.0, None, TT.is_ge)
        le127 = ttile()
        nc.vector.tensor_scalar(le127[:], x0[:], 127.0, None, TT.is_le)
        le126 = ttile()
        nc.vector.tensor_scalar(le126[:], x0[:], 126.0, None, TT.is_le)
        eqm1 = ttile()
        nc.vector.tensor_scalar(eqm1[:], x0[:], -1.0, None, TT.is_equal)
        # c0 = wx0*inb(x0) + wx1*(x0 == -1);  c1 = wx1*(0 <= x0 <= 126)
        c0 = ttile()
        nc.vector.tensor_tensor(c0[:], ge0[:], le127[:], TT.mult)
        nc.vector.tensor_tensor(c0[:], c0[:], wx0[:], TT.mult)
        t_ = ttile()
        nc.vector.tensor_tensor(t_[:], wx1[:], eqm1[:], TT.mult)
        nc.vector.tensor_tensor(c0[:], c0[:], t_[:], TT.add)
        c1 = ttile()
        nc.vector.tensor_tensor(c1[:], ge0[:], le126[:], TT.mult)
        nc.vector.tensor_tensor(c1[:], c1[:], wx1[:], TT.mult)
        # xs = clip(x0, 0, 127); kh = floor(xs/2); s = xs - 2*kh
        xs = ttile()
        nc.vector.tensor_scalar(xs[:], x0[:], 0.0, 127.0, TT.max, TT.min)
        xh = ttile()
        nc.vector.tensor_scalar(xh[:], xs[:], 0.5, None, TT.mult)
        kh = _floor(nc, wm, mybir, xh, S8, "fkh")
        spar = ttile()
        nc.vector.scalar_tensor_tensor(spar[:], kh[:], -2.0, xs[:], TT.mult, TT.add)

        # --- y chain ---
        py = ttile()
        nc.vector.tensor_scalar(py[:], view_off(1), 1.0 / GRID, None, TT.mult)
        nc.vector.tensor_tensor(
            py[:], py[:], refs[:, :, 1:2].broadcast_to(S8), TT.add)
        nc.vector.tensor_scalar(py[:], py[:], float(GRID), -0.5, TT.mult, TT.add)
        y0 = _floor(nc, wm, mybir, py, S8, "fy0")
        wy1 = ttile()
        nc.vector.tensor_tensor(wy1[:], py[:], y0[:], TT.subtract)
        wy0m = ttile()
        nc.vector.tensor_scalar(wy0m[:], wy1[:], -1.0, 1.0, TT.mult, TT.add)
        yge0 = ttile()
        nc.vector.tensor_scalar(yge0[:], y0[:], 0.0, None, TT.is_ge)
        yle127 = ttile()
        nc.vector.tensor_scalar(yle127[:], y0[:], 127.0, None, TT.is_le)
        nc.vector.tensor_tensor(yge0[:], yge0[:], yle127[:], TT.mult)
        nc.vector.tensor_tensor(wy0m[:], wy0m[:], yge0[:], TT.mult)  # wy0*inb(y0)
        ygem1 = ttile()
        nc.vector.tensor_scalar(ygem1[:], y0[:], -1.0, None, TT.is_ge)
        yle126 = ttile()
        nc.vector.tensor_scalar(yle126[:], y0[:], 126.0, None, TT.is_le)
        nc.vector.tensor_tensor(ygem1[:], ygem1[:], yle126[:], TT.mult)
        nc.vector.tensor_tensor(wy1[:], wy1[:], ygem1[:], TT.mult)   # wy1*inb(y1)
        r0 = ttile()
        nc.vector.tensor_scalar(r0[:], y0[:], 0.0, 127.0, TT.max, TT.min)
        r1 = ttile()
        nc.vector.tensor_scalar(r1[:], y0[:], 1.0, None, TT.add)
        nc.vector.tensor_scalar(r1[:], r1[:], 0.0, 127.0, TT.max, TT.min)

        # --- softmax over pt ---
        logit4 = off_all[:, :, 64:96].rearrange("p g (h pt) -> p g h pt", pt=4)
        mx = wm.tile([128, 8, 8], F32, name="smx", tag="smx")
        nc.vector.tensor_reduce(mx[:], logit4, mybir.AxisListType.X, TT.max)
        ee = ttile()
        nc.vector.tensor_tensor(
            ee[:].rearrange("p g (h pt) -> p g h pt", pt=4), logit4,
            mx[:].unsqueeze(3).broadcast_to([128, 8, 8, 4]),
            TT.subtract)
        nc.scalar.activation(ee[:], ee[:], ACTF.Exp)
        ssum = wm.tile([128, 8, 8], F32, name="ssum", tag="ssum")
        nc.vector.tensor_reduce(
            ssum[:], ee[:].rearrange("p g (h pt) -> p g h pt", pt=4),
            mybir.AxisListType.X, TT.add)
        rec = wm.tile([128, 8, 8], F32, name="srec", tag="srec")
        nc.vector.reciprocal(rec[:], ssum[:])
        attn = ttile()
        nc.vector.tensor_tensor(
            attn[:].rearrange("p g (h pt) -> p g h pt", pt=4),
            ee[:].rearrange("p g (h pt) -> p g h pt", pt=4),
            rec[:].unsqueeze(3).broadcast_to([128, 8, 8, 4]),
            TT.mult)

        # --- fused coefficients C[y][j] = attn * wy_y_masked * v_j ---
        # v0 = c0*(s==0), v1 = c0*(s==1)+c1*(s==0), v2 = c1*(s==1)
        p0 = ttile()
        nc.vector.tensor_scalar(p0[:], spar[:], 0.0, None, TT.is_equal)
        p1 = ttile()
        nc.vector.tensor_scalar(p1[:], spar[:], 1.0, None, TT.is_equal)
        g0 = ttile()
        nc.vector.tensor_tensor(g0[:], attn[:], wy0m[:], TT.mult)
        g1 = ttile()
        nc.vector.tensor_tensor(g1[:], attn[:], wy1[:], TT.mult)
        v0 = ttile()
        nc.vector.tensor_tensor(v0[:], c0[:], p0[:], TT.mult)
        v1 = ttile()
        nc.vector.tensor_tensor(v1[:], c0[:], p1[:], TT.mult)
        nc.vector.tensor_tensor(t_[:], c1[:], p0[:], TT.mult)
        nc.vector.tensor_tensor(v1[:], v1[:], t_[:], TT.add)
        v2 = ttile()
        nc.vector.tensor_tensor(v2[:], c1[:], p1[:], TT.mult)
        # coeff tensor [128, qtop, y, j, h, pt] fp32
        coef = cpool.tile([128, 8, 2, 3, 8, 4], F32)
        for yi, gy in ((0, g0), (1, g1)):
            for ji, vj in ((0, v0), (1, v1), (2, v2)):
                nc.vector.tensor_tensor(
                    coef[:, :, yi, ji, :, :],
                    gy[:].rearrange("p g (h pt) -> p g h pt", pt=4),
                    vj[:].rearrange("p g (h pt) -> p g h pt", pt=4), TT.mult)

        # bias-fold factor S[q, h] = sum_pt attn*(wy0m+wy1m)*(c0+c1)
        wys = ttile()
        nc.vector.tensor_tensor(wys[:], wy0m[:], wy1[:], TT.add)
        cxs = ttile()
        nc.vector.tensor_tensor(cxs[:], c0[:], c1[:], TT.add)
        nc.vector.tensor_tensor(wys[:], wys[:], cxs[:], TT.mult)
        nc.vector.tensor_tensor(wys[:], wys[:], attn[:], TT.mult)
        sfac = cpool.tile([128, 8, 8], F32)
        nc.vector.tensor_reduce(
            sfac[:], wys[:].rearrange("p g (h pt) -> p g h pt", pt=4),
            mybir.AxisListType.X, TT.add)

        # --- gather window indices idxf [128, (qtop, h, pt, y)] fp32 ---
        idxf = cpool.tile([128, 8, 8, 4, 2], F32)
        for yi, rr in ((0, r0), (1, r1)):
            nc.vector.scalar_tensor_tensor(
                idxf[:, :, :, :, yi],
                rr[:].rearrange("p g (h pt) -> p g h pt", pt=4), 64.0,
                kh[:].rearrange("p g (h pt) -> p g h pt", pt=4),
                TT.mult, TT.add)

        # ---------- idx layout transform: [128, 512] -> [16, 64, 64] ----------
        # T1: 4 PE transposes -> T-all [4][128 f, 128 q%128]
        tall = [cpool.tile([128, 128], F32, name=f"tall{t}", tag=f"tall{t}")
                for t in range(4)]
        idxf_flat = idxf[:].rearrange("p g h pt y -> p (g h pt y)")
        for t in range(4):
            pt_ = ps_tr()
            nc.tensor.transpose(
                pt_[:], idxf_flat[:, t * 128:(t + 1) * 128], ident[:])
            nc.scalar.copy(tall[t][:], pt_[:])
        # T2: per (t, qmid): [128 f, 16] -> [16, 128 f]; scatter into IDX
        idx_f2 = cpool.tile([16, 64, 8, 8], F32)  # [qlo, call, qtop, qmid]
        for t in range(4):
            for qmid in range(8):
                ptf = ps_tr()
                pt_ = ptf[0:16, :]
                nc.tensor.transpose(
                    pt_, tall[t][:, qmid * 16:qmid * 16 + 16], ident[:])
                # f = t*128 + j, j = (qtop%2)*64 + call ; qtop = 2t + (j//64)
                src = pt_.rearrange("a (q2 c) -> a q2 c", q2=2)
                nc.scalar.copy(
                    idx_f2[:, :, 2 * t:2 * t + 2, qmid].rearrange(
                        "a c q2 -> a q2 c"), src)
        idx16 = cpool.tile([16, 64 * 64], I16)
        nc.vector.tensor_copy(
            idx16[:], idx_f2[:].rearrange("a c g q -> a (c g q)"))
        idxr = cpool.tile([128, 64, 64], I16)
        for rep in range(8):
            nc.sync.dma_start(
                idxr[rep * 16:(rep + 1) * 16, :, :],
                idx16[:].rearrange("a (c b) -> a c b", c=64))

        # ---------- 3. value projection ----------
        w_val = cpool.tile([128, 2, C], BF16)
        nc.sync.dma_start(w_val[:], w_value_e[:].rearrange(
            "(k p) o -> p k o", k=2))
        zpad = cpool.tile([2, 64], BF16)
        nc.vector.memset(zpad[:], 0.0)
        for pr in range(4):
            nc.sync.dma_start(value_d[pr, HW:HW + 2, :], zpad[:])

        for mc in range(128):
            mt = mpool.tile([128, C], BF16)
            nc.gpsimd.dma_start(mt[:], memory_e[mc * 128:(mc + 1) * 128, :])
            mT = mpool.tile([128, 2, 128], BF16, tag="mT")
            for kc in range(2):
                pt_ = ps_trb()
                nc.tensor.transpose(
                    pt_[:], mt[:, kc * 128:(kc + 1) * 128], identb[:])
                nc.scalar.copy(mT[:, kc, :], pt_[:])
            pv = ps_mm()
            for kc in range(2):
                nc.tensor.matmul(pv[:], mT[:, kc, :], w_val[:, kc, :],
                                 start=(kc == 0), stop=(kc == 1))
            vt = vpool.tile([128, C], BF16)
            nc.scalar.copy(vt[:], pv[:])
            for pr in range(4):
                eng = nc.sync if (mc * 4 + pr) % 2 == 0 else nc.scalar
                eng.dma_start(
                    value_d[pr, mc * 128:(mc + 1) * 128, :],
                    vt[:, pr * 64:(pr + 1) * 64])

        # ---------- 4 & 5. gather + blend ----------
        sampled = cpool.tile([128, 8, 8, 32], F32)  # [q%128, qtop, h, c]
        val_flat = value_d[:].rearrange("pr r c -> (pr r c)")
        for h in range(NH):
            pr = h // 2
            half = h % 2
            base = pr * (PAIR_ROWS * 64)
            in_ap = val_flat[base:base + 8192 * 128].rearrange(
                "(n c) -> n c", c=128).copy()
            in_ap.ap[-1] = (1, 256)  # overlapping 256-elem windows, step 128
            acc = spool.tile([128, 8, 32], F32, tag="acc")
            first = True
            for pt_i in range(NPT):
                for yi in range(2):
                    call = ((h * NPT) + pt_i) * 2 + yi
                    gat = gpool.tile([128, 8, 256], BF16)
                    nc.gpsimd.dma_gather(
                        gat[:], in_ap, idxr[:, call, :], LQ, LQ, 256,
                        elem_step=128)
                    sc = gpool.tile([128, 8, 3, 32], F32, tag="scaled")
                    g3 = gat[:].rearrange("p g (j c) -> p g j c", c=64)[
                        :, :, 0:3, half * 32:half * 32 + 32]
                    cf = coef[:, :, yi, :, h, pt_i].unsqueeze(3).broadcast_to([128, 8, 3, 32])
                    nc.vector.tensor_tensor(sc[:], g3, cf, TT.mult)
                    red = gpool.tile([128, 8, 32], F32, tag="red")
                    nc.vector.tensor_reduce(
                        red[:], sc[:].rearrange("p g j c -> p g c j"),
                        mybir.AxisListType.X, TT.add)
                    if first:
                        nc.vector.tensor_copy(acc[:], red[:])
                        first = False
                    else:
                        nc.vector.tensor_tensor(acc[:], acc[:], red[:], TT.add)
            nc.vector.tensor_copy(sampled[:, :, h, :], acc[:])

        # bias fold: sampled += S[q, h] * b_value[h*32 + c]
        b_val = cpool.tile([128, C], F32)
        nc.sync.dma_start(b_val[:], b_val_e[:])
        bterm = spool.tile([128, 8, 8, 32], F32, tag="bterm")
        nc.vector.tensor_tensor(
            bterm[:],
            sfac[:].unsqueeze(3).broadcast_to([128, 8, 8, 32]),
            b_val[:].rearrange("p (h c) -> p h c", h=8).unsqueeze(1).broadcast_to(
                [128, 8, 8, 32]),
            TT.mult)
        nc.vector.tensor_tensor(sampled[:], sampled[:], bterm[:], TT.add)

        # ---------- 6. output projection ----------
        # sampledT [2][128 hc, (qtop, q%128)] f32r
        sT = [cpool.tile([128, 8, 128], F32R, name=f"sT{i}", tag=f"sT{i}")
              for i in range(2)]
        for qt_ in range(8):
            for hf in range(2):
                pt_ = ps_tr()
                nc.tensor.transpose(
                    pt_[:],
                    sampled[:, qt_, hf * 4:(hf + 1) * 4, :].rearrange(
                        "p h c -> p (h c)"),
                    ident[:])
                nc.scalar.copy(sT[hf][:, qt_, :], pt_[:])
        w_out = cpool.tile([128, 2, C], F32R)
        nc.sync.dma_start(w_out[:], w_out_e[:].rearrange(
            "(k p) o -> p k o", k=2))
        b_out = cpool.tile([128, C], F32)
        nc.sync.dma_start(b_out[:], b_out_e[:])
        for qt_ in range(8):
            po = ps_mm()
            for kc in range(2):
                nc.tensor.matmul(po[:], sT[kc][:, qt_, :], w_out[:, kc, :],
                                 start=(kc == 0), stop=(kc == 1))
            ot = qpool.tile([128, C], F32, tag="out")
            nc.vector.tensor_tensor(ot[:], po[:], b_out[:], TT.add)
            nc.sync.dma_start(out_e[qt_ * 128:(qt_ + 1) * 128, :], ot[:])

    nc.finalize()
    return nc


_CACHE = {}


def _get_program():
    if "nc" not in _CACHE:
        _CACHE["nc"] = build_program()
    return _CACHE["nc"]


def run(inputs, trace=False):
    from concourse.bass_utils import run_bass_kernel_spmd

    nc = _get_program()
    query = np.asarray(inputs["query"], np.float32)
    memory = np.asarray(inputs["memory"], np.float32)
    refpts = np.asarray(inputs["reference_points"], np.float32)
    w_value = np.asarray(inputs["W_value"], np.float32).astype(ml_dtypes.bfloat16)
    b_value = np.asarray(inputs["b_value"], np.float32)
    w_off = np.asarray(inputs["W_off"], np.float32)
    b_off = np.asarray(inputs["b_off"], np.float32)
    w_attn = np.asarray(inputs["W_attn"], np.float32)
    b_attn = np.asarray(inputs["b_attn"], np.float32)
    w_out = np.asarray(inputs["W_out"], np.float32)
    b_out = np.asarray(inputs["b_out"], np.float32)

    w_oa = np.concatenate([w_off, w_attn], axis=1).astype(np.float32)
    b_oa = np.tile(np.concatenate([b_off, b_attn])[None, :], (128, 1)).astype(
        np.float32)
    b_out_r = np.tile(b_out[None, :], (128, 1)).astype(np.float32)
    b_val_r = np.tile(b_value[None, :], (128, 1)).astype(np.float32)
    ident = np.eye(128, dtype=np.float32)
    identb = np.eye(128, dtype=ml_dtypes.bfloat16)

    shared = dict(w_value=w_value, w_oa=w_oa, b_oa=b_oa, w_out=w_out,
                  b_out=b_out_r, b_val=b_val_r, ident=ident, identb=identb)
    in_maps = []
    for i in range(N_B):
        m = dict(shared)
        m["query"] = query[i]
        m["memory"] = memory[i]
        m["refpts"] = refpts[i]
        in_maps.append(m)

    res = run_bass_kernel_spmd(nc, in_maps, list(range(N_B)), trace=trace,
                               trace_cores=[0])
    out = np.stack([res.results[i]["out"] for i in range(N_B)], axis=0)
    return out, res


def kernel(**inputs):
    assert int(inputs.get("H", GRID)) == GRID and int(inputs.get("W", GRID)) == GRID
    out, _ = run(inputs, trace=False)
    return out.astype(np.float32)

